# revision 30
# baseline (speedup 1.0000x reference)
"""Trainium2 Bass kernel for CodecAttention (GQA + full-width RMSNorm + ALiBi
+ 512 sliding causal window), SPMD over 8 NeuronCores.

Sharding: 2 batches x 4 sequence chunks of 512 queries per core. Each core
sees a [q0-512, q0+512) feature slice (zero-padded left halo for chunk 0),
computes its own QKV projections + norms + banded windowed attention + output
projection. Host only slices/transposes inputs and concatenates outputs.
"""

import math

import numpy as np

import concourse.bass as bass
import concourse.tile as tile
from concourse import bacc, mybir

F32 = mybir.dt.float32
F32R = mybir.dt.float32r
AF = mybir.ActivationFunctionType

# Problem constants (hardcoded per spec nn_CodecAttention_34308198761010)
B, S, M = 2, 2048, 1024
H, KV, D = 16, 4, 64
WIN = 512
SQ = 512          # queries per core
SK = 1024         # k-range per core (halo + chunk)
NCHUNK = S // SQ  # 4
N_CORES = 8
EPS = 1e-6
# heads 0..AUG_HEADS-1 get a per-query exponent correction -slope*q accumulated
# into scores (rank-1 matmul) so exp() stays bounded; for flatter slopes the
# uncorrected exponent (<= slope*511 + ~8) fits comfortably in f32.
AUG_HEADS = 6
# Head order permutation (host-side, applied to wq/q_norm_w/wo): Q-head slot
# 2j+sub sits at partition offset 64*sub, which must equal its KV head's
# partition offset 64*((h//4)%2) (matmul requires equal base partitions).
PERM = [0, 4, 1, 5, 2, 6, 3, 7, 8, 12, 9, 13, 10, 14, 11, 15]

# banded spans: for k-tile kt (128 rows of kl), valid q columns [QL[kt], QR[kt])
QL = [0, 0, 0, 0, 0, 128, 256, 384]
QR = [128, 256, 384, 512, 512, 512, 512, 512]


def _alibi_slopes(n):
    ratio = 2.0 ** (-8.0 / n)
    return np.array([ratio**i for i in range(n)], dtype=np.float64)


def build_nc(for_sim=False):
    nc = bacc.Bacc(None, target_bir_lowering=False, debug=for_sim)

    feat_d = nc.dram_tensor("feat", [128, 8, SK], F32R, kind="ExternalInput")
    wq_d = nc.dram_tensor("wq", [8, 128, 8, 128], F32R, kind="ExternalInput")
    wk_d = nc.dram_tensor("wk", [128, 8, 256], F32R, kind="ExternalInput")
    wv_d = nc.dram_tensor("wv", [128, 8, 256], F32R, kind="ExternalInput")
    wo_d = nc.dram_tensor("wo", [128, 8, M], F32R, kind="ExternalInput")
    qw_d = nc.dram_tensor("qw", [128, 8], F32, kind="ExternalInput")
    kw_d = nc.dram_tensor("kw", [128, 2], F32, kind="ExternalInput")
    btab_d = nc.dram_tensor("btab", [128, H * 8], F32, kind="ExternalInput")
    tri_d = nc.dram_tensor("tri", [128, 2, 128], F32, kind="ExternalInput")
    qaug_d = nc.dram_tensor("qaug", [1, 16, SQ], F32R, kind="ExternalInput")
    ones_d = nc.dram_tensor("onesin", [128, 128], F32R, kind="ExternalInput")
    vone_d = nc.dram_tensor("vone", [128, 8, KV, 2], F32R, kind="ExternalInput")
    out_d = nc.dram_tensor("out", [SQ, M], F32, kind="ExternalOutput")

    with tile.TileContext(nc) as tc:
        with (
            nc.allow_low_precision("f32r matmul operands are intentional"),
            tc.tile_pool(name="const", bufs=1) as constp,
            tc.tile_pool(name="feat", bufs=1) as featp,
            tc.tile_pool(name="wbig", bufs=1) as wbigp,
            tc.tile_pool(name="wstream", bufs=2) as wsp,
            tc.tile_pool(name="acts", bufs=1) as actsp,
            tc.tile_pool(name="sexp", bufs=3) as sexpp,
            tc.tile_pool(name="sqpool", bufs=2) as sqp,
            tc.tile_pool(name="small", bufs=1) as smallp,
            tc.tile_pool(name="outsb", bufs=2) as outp,
        ):
            # ---- constants / inputs ----
            feat_sb = featp.tile([128, 8, SK], F32R)
            for c in range(4):
                nc.sync.dma_start(feat_sb[:, 2 * c:2 * c + 2, :],
                                  feat_d[:, 2 * c:2 * c + 2, :])
            wk_sb = constp.tile([128, 8, 256], F32R)
            nc.sync.dma_start(wk_sb[:], wk_d[:])
            wv_sb = constp.tile([128, 8, 256], F32R)
            nc.sync.dma_start(wv_sb[:], wv_d[:])
            wo_sb = wbigp.tile([128, 8, M], F32R)
            for c in range(4):
                nc.sync.dma_start(wo_sb[:, 2 * c:2 * c + 2, :],
                                  wo_d[:, 2 * c:2 * c + 2, :])
            qw_sb = constp.tile([128, 8], F32)
            nc.sync.dma_start(qw_sb[:], qw_d[:])
            kw_sb = constp.tile([128, 2], F32)
            nc.sync.dma_start(kw_sb[:], kw_d[:])
            btab_sb = constp.tile([128, H * 8], F32)
            nc.sync.dma_start(btab_sb[:], btab_d[:])
            tri_sb = constp.tile([128, 2, 128], F32)
            nc.sync.dma_start(tri_sb[:], tri_d[:])
            qaug_sb = constp.tile([1, 16, SQ], F32R)
            nc.sync.dma_start(qaug_sb[:], qaug_d[:])

            ones_sb = constp.tile([128, 128], F32R)
            nc.sync.dma_start(ones_sb[:], ones_d[:])
            eps_sb = constp.tile([128, 1], F32)
            nc.vector.memset(eps_sb[:], EPS)

            # ---- K projection + rmsnorm (KT [kv_hd, SK] as [128, 2, SK]) ----
            kt_sb = actsp.tile([128, 2, SK], F32R)
            with (
                tc.tile_pool(name="psK", bufs=2, space=bass.MemorySpace.PSUM) as psK,
                tc.tile_pool(name="psSK", bufs=1, space=bass.MemorySpace.PSUM) as psSK,
                tc.tile_pool(name="psBK", bufs=1, space=bass.MemorySpace.PSUM) as psBK,
            ):
                ssqk = psSK.tile([1, SK], F32)
                for t in range(2):
                    kp = psK.tile([128, SK], F32)
                    for half in range(2):
                        cs = slice(512 * half, 512 * half + 512)
                        for mt in range(8):
                            nc.tensor.matmul(
                                kp[:, cs],
                                wk_sb[:, mt, 128 * t:128 * t + 128],
                                feat_sb[:, mt, cs],
                                start=(mt == 0), stop=(mt == 7))
                    for half in range(2):
                        cs = slice(512 * half, 512 * half + 512)
                        sqk = sqp.tile([128, 512], F32R, tag="sqk")
                        nc.scalar.activation(sqk[:], kp[:, cs], AF.Square)
                        nc.tensor.matmul(ssqk[0:1, cs], ones_sb[:, 0:1],
                                         sqk[:],
                                         start=(t == 0), stop=(t == 1))
                    # kraw = k * k_norm_w  (per-partition scale)
                    nc.scalar.activation(kt_sb[:, t, :], kp[:], AF.Copy,
                                         scale=kw_sb[:, t:t + 1])
                # rsqrt(mean + eps) per column
                srtk = smallp.tile([1, SK], F32R, tag="srtk")
                nc.scalar.activation(srtk[:], ssqk[:], AF.Sqrt,
                                     scale=1.0 / 256.0, bias=eps_sb[0:1, :])
                nc.vector.reciprocal(srtk[:], srtk[:])
                bck = psBK.tile([128, SK], F32)
                for half in range(2):
                    cs = slice(512 * half, 512 * half + 512)
                    nc.tensor.matmul(bck[:, cs], ones_sb[0:1, :],
                                     srtk[0:1, cs], start=True, stop=True)
                bck_sb = smallp.tile([128, SK], F32, tag="bck")
                nc.vector.tensor_copy(bck_sb[:], bck[:])
                for t in range(2):
                    nc.vector.tensor_mul(kt_sb[:, t, :], kt_sb[:, t, :], bck_sb[:])

            # ---- V projection (natural [s, kv_hd] layout, with ones cols) ----
            # v_sb[p, st, g, 0] = 1, [.., 1:65] = V, [.., 65] = 1
            v_sb = actsp.tile([128, 8, KV, 66], F32R)
            nc.sync.dma_start(v_sb[:, :, :, 0:1], vone_d[:, :, :, 0:1])
            nc.sync.dma_start(v_sb[:, :, :, 65:66], vone_d[:, :, :, 1:2])
            with tc.tile_pool(name="psV", bufs=3, space=bass.MemorySpace.PSUM) as psV:
                for st in range(8):
                    vp = psV.tile([128, 256], F32)
                    for mt in range(8):
                        nc.tensor.matmul(
                            vp[:],
                            feat_sb[:, mt, 128 * st:128 * st + 128],
                            wv_sb[:, mt, :],
                            start=(mt == 0), stop=(mt == 7))
                    nc.scalar.activation(
                        v_sb[:, st, :, 1:65],
                        vp[:].rearrange("p (g d) -> p g d", g=KV),
                        AF.Copy)

            # ---- Q projection + rmsnorm (QT [hd, SQ] as [128, 8, SQ]) ----
            qt_sb = actsp.tile([128, 8, SQ], F32R)
            with (
                tc.tile_pool(name="psQ", bufs=3, space=bass.MemorySpace.PSUM) as psQ,
                tc.tile_pool(name="psSQ", bufs=1, space=bass.MemorySpace.PSUM) as psSQ,
                tc.tile_pool(name="psBQ", bufs=1, space=bass.MemorySpace.PSUM) as psBQ,
            ):
                ssqq = psSQ.tile([1, SQ], F32)
                for t in range(8):
                    wqt = wsp.tile([128, 8, 128], F32R, tag="wqt")
                    nc.sync.dma_start(wqt[:], wq_d[t])
                    qp = psQ.tile([128, SQ], F32)
                    for mt in range(8):
                        nc.tensor.matmul(qp[:], wqt[:, mt, :],
                                         feat_sb[:, mt, 512:1024],
                                         start=(mt == 0), stop=(mt == 7))
                    sqq = sqp.tile([128, SQ], F32R, tag="sqk")
                    nc.scalar.activation(sqq[:], qp[:], AF.Square)
                    nc.tensor.matmul(ssqq[:], ones_sb[:, 0:1], sqq[:],
                                     start=(t == 0), stop=(t == 7))
                    nc.scalar.activation(qt_sb[:, t, :], qp[:], AF.Copy,
                                         scale=qw_sb[:, t:t + 1])
                srtq = smallp.tile([1, SQ], F32R, tag="srtk")
                nc.scalar.activation(srtq[:], ssqq[:], AF.Sqrt,
                                     scale=1.0 / 1024.0, bias=eps_sb[0:1, :])
                nc.vector.reciprocal(srtq[:], srtq[:])
                bcq = psBQ.tile([128, SQ], F32)
                nc.tensor.matmul(bcq[:], ones_sb[0:1, :], srtq[0:1, :],
                                 start=True, stop=True)
                bcq_sb = smallp.tile([128, SQ], F32, tag="bck")
                nc.vector.tensor_copy(bcq_sb[:], bcq[:])
                for t in range(8):
                    nc.vector.tensor_mul(qt_sb[:, t, :], qt_sb[:, t, :], bcq_sb[:])

            # ---- attention (per head pair), banded sliding window ----
            attn_sb = actsp.tile([128, 8, SQ], F32R)
            KT_ORDER = [3, 0, 1, 2, 4, 5, 6, 7]
            with (
                tc.tile_pool(name="psS", bufs=2, space=bass.MemorySpace.PSUM) as psS,
                tc.tile_pool(name="psPV", bufs=2, space=bass.MemorySpace.PSUM) as psPV,
                tc.tile_pool(name="psBC", bufs=1, space=bass.MemorySpace.PSUM) as psBC,
            ):
                for pair in range(8):
                    pv_e = psPV.tile([128, SQ], F32, tag="pve")
                    pv_o = psPV.tile([128, SQ], F32, tag="pvo")
                    for sub in range(2):
                        h = PERM[2 * pair + sub]
                        g = h // 4
                        po = 64 * (g % 2)
                        qo = 64 * sub
                        assert po == qo
                        for i, kt in enumerate(KT_ORDER):
                            ql, qr = QL[kt], QR[kt]
                            span = qr - ql
                            sp = psS.tile([128, SQ], F32, tag="scores")
                            aug = h < AUG_HEADS
                            nc.tensor.matmul(
                                sp[:, 0:span],
                                kt_sb[po:po + 64, g // 2,
                                         128 * kt:128 * kt + 128],
                                qt_sb[qo:qo + 64, pair, ql:qr],
                                start=True, stop=not aug)
                            if aug:
                                nc.tensor.matmul(
                                    sp[:, 0:span],
                                    ones_sb[0:1, 0:128],
                                    qaug_sb[0:1, h, ql:qr],
                                    start=False, stop=True)
                            # additive -8e30 triangle masks, BEFORE exp
                            if kt <= 3:  # right triangle: cols [128kt, +128)
                                nc.vector.tensor_add(
                                    sp[:, 128 * kt:128 * kt + 128],
                                    sp[:, 128 * kt:128 * kt + 128],
                                    tri_sb[:, 1, :])
                            else:  # left triangle: first 128 cols of span
                                nc.vector.tensor_add(
                                    sp[:, 0:128], sp[:, 0:128], tri_sb[:, 0, :])
                            se = sexpp.tile([128, SQ], F32R, tag="sexp")
                            c = h * 8 + kt
                            nc.scalar.activation(se[:, 0:span], sp[:, 0:span],
                                                 AF.Exp,
                                                 bias=btab_sb[:, c:c + 1],
                                                 scale=0.125)
                            # [V | ones]: rows 0:64 = PV, row 64 = denom
                            # (matmul out must start at partition 0)
                            pv = pv_e if sub == 0 else pv_o
                            nc.tensor.matmul(
                                pv[0:65, ql:qr],
                                v_sb[:, kt, g, 1:66],
                                se[:, 0:span],
                                start=(i == 0), stop=(i == 7))
                    # normalize: attn = pv / denom
                    rec = smallp.tile([128, 2, SQ], F32R, tag="rec")
                    nc.vector.reciprocal(rec[64:65, 0, :], pv_e[64:65, :])
                    nc.vector.reciprocal(rec[64:65, 1, :], pv_o[64:65, :])
                    bc = psBC.tile([64, 2, SQ], F32)
                    nc.tensor.matmul(bc[0:64, 0, :], ones_sb[64:65, 0:64],
                                     rec[64:65, 0, :], start=True, stop=True)
                    nc.tensor.matmul(bc[0:64, 1, :], ones_sb[64:65, 0:64],
                                     rec[64:65, 1, :], start=True, stop=True)
                    bc_sb = smallp.tile([64, 2, SQ], F32, tag="bcsb")
                    nc.vector.tensor_copy(bc_sb[:], bc[:])
                    nc.vector.tensor_mul(attn_sb[0:64, pair, :],
                                         pv_e[0:64, :], bc_sb[:, 0, :])
                    attn_tmp = smallp.tile([64, SQ], F32R, tag="attn_tmp")
                    nc.vector.tensor_mul(attn_tmp[:],
                                         pv_o[0:64, :], bc_sb[:, 1, :])
                    # partition shift 0:64 -> 64:128 via SBUF-to-SBUF DMA
                    nc.sync.dma_start(attn_sb[64:128, pair, :], attn_tmp[:])

            # ---- output projection: out[s, m] ----
            with tc.tile_pool(name="psO", bufs=3, space=bass.MemorySpace.PSUM) as psO:
                for st in range(4):
                    osb = outp.tile([128, M], F32)
                    for mh in range(2):
                        op = psO.tile([128, 512], F32)
                        for ht in range(8):
                            nc.tensor.matmul(
                                op[:],
                                attn_sb[:, ht, 128 * st:128 * st + 128],
                                wo_sb[:, ht, 512 * mh:512 * mh + 512],
                                start=(ht == 0), stop=(ht == 7))
                        nc.vector.tensor_copy(osb[:, 512 * mh:512 * mh + 512],
                                              op[:])
                    nc.sync.dma_start(
                        out_d.rearrange("(st p) m -> st p m", p=128)[st],
                        osb[:])

    if for_sim:
        nc.compile()
    else:
        nc.finalize()
    return nc


def make_in_maps(features, wq, wk, wv, wo, q_norm_w, k_norm_w):
    features = np.asarray(features, np.float32)
    wq = np.asarray(wq, np.float32)
    wk = np.asarray(wk, np.float32)
    wv = np.asarray(wv, np.float32)
    wo = np.asarray(wo, np.float32)
    q_norm_w = np.asarray(q_norm_w, np.float32)
    k_norm_w = np.asarray(k_norm_w, np.float32)

    # permute Q-head order (see PERM) in wq rows, q_norm_w, wo columns
    wq_p = wq.reshape(H, D, M)[PERM].reshape(H * D, M)
    qnw_p = q_norm_w.reshape(H, D)[PERM].reshape(H * D)
    wo_tp = wo.T.reshape(H, D, M)[PERM].reshape(H * D, M)  # wo.T rows = hd

    wq_pre = np.ascontiguousarray(
        wq_p.T.reshape(8, 128, 8, 128).transpose(2, 1, 0, 3))
    # wq_p.T is [m, hd]; [mt,p,ht,c] -> want [ht, p(m%128), mt, c]
    wk_pre = np.ascontiguousarray(wk.T.reshape(8, 128, 256).transpose(1, 0, 2))
    wv_pre = np.ascontiguousarray(wv.T.reshape(8, 128, 256).transpose(1, 0, 2))
    wo_pre = np.ascontiguousarray(wo_tp.reshape(8, 128, M).transpose(1, 0, 2))
    qw_pre = np.ascontiguousarray(qnw_p.reshape(8, 128).T)
    kw_pre = np.ascontiguousarray(k_norm_w.reshape(2, 128).T)

    slopes = _alibi_slopes(H)
    p = np.arange(128)
    qaug = np.zeros((1, 16, SQ), np.float32)
    qi = np.arange(SQ, dtype=np.float64)
    for h in range(AUG_HEADS):
        qaug[0, h, :] = -8.0 * slopes[h] * qi
    # additive masks: 0 where valid, -8e30 where invalid (applied pre-exp)
    tri = np.zeros((128, 2, 128), np.float32)
    cc = np.arange(128)
    tri[:, 0, :] = np.where(cc[None, :] >= p[:, None], 0.0, -8e30)
    tri[:, 1, :] = np.where(cc[None, :] <= p[:, None], 0.0, -8e30)

    in_maps = []
    for b in range(B):
        for c in range(NCHUNK):
            q0 = c * SQ
            lo, hi = q0 - WIN, q0 + SQ
            fs = np.zeros((SK, M), np.float32)
            src_lo = max(lo, 0)
            fs[src_lo - lo:, :] = features[b, src_lo:hi, :]
            feat_pre = np.ascontiguousarray(
                fs.T.reshape(8, 128, SK).transpose(1, 0, 2))
            btab = np.zeros((128, H * 8), np.float32)
            for h in range(H):
                for kt in range(8):
                    if c == 0 and kt < 4:
                        btab[:, h * 8 + kt] = -1e30
                    else:
                        btab[:, h * 8 + kt] = slopes[h] * (128 * kt + p - 512)
            in_maps.append({
                "feat": feat_pre, "wq": wq_pre, "wk": wk_pre, "wv": wv_pre,
                "wo": wo_pre, "qw": qw_pre, "kw": kw_pre,
                "btab": btab, "tri": tri, "qaug": qaug,
                "onesin": np.ones((128, 128), np.float32),
                "vone": np.ones((128, 8, KV, 2), np.float32),
            })
    return in_maps


_NC_CACHE = {}


def kernel(features, wq, wk, wv, wo, q_norm_w, k_norm_w,
           num_heads=16, num_kv_heads=4, head_dim=64, sliding_window=512,
           **_unused):
    assert int(num_heads) == H and int(num_kv_heads) == KV
    assert int(head_dim) == D and int(sliding_window) == WIN
    from concourse.bass_utils import run_bass_kernel_spmd

    if "nc" not in _NC_CACHE:
        _NC_CACHE["nc"] = build_nc(for_sim=False)
    nc = _NC_CACHE["nc"]
    in_maps = make_in_maps(features, wq, wk, wv, wo, q_norm_w, k_norm_w)
    res = run_bass_kernel_spmd(nc, in_maps, core_ids=list(range(N_CORES)))
    outs = [r["out"] for r in res.results]
    full = np.stack(outs, axis=0).reshape(B, NCHUNK * SQ, M)
    return full.astype(np.float32)


# revision 39
# speedup vs baseline: 1.4167x; 1.4167x over previous
"""Trainium2 Bass kernel for CodecAttention (GQA + full-width RMSNorm + ALiBi
+ 512 sliding causal window), SPMD over 8 NeuronCores.

Sharding: 2 batches x 4 sequence chunks of 512 queries per core. Each core
sees a [q0-512, q0+512) feature slice (zero-padded left halo for chunk 0),
computes its own QKV projections + norms + banded windowed attention + output
projection. Host only slices/transposes inputs and concatenates outputs.
"""

import math

import numpy as np

import concourse.bass as bass
import concourse.tile as tile
from concourse import bacc, mybir

F32 = mybir.dt.float32
F32R = mybir.dt.float32r
AF = mybir.ActivationFunctionType

# Problem constants (hardcoded per spec nn_CodecAttention_34308198761010)
B, S, M = 2, 2048, 1024
H, KV, D = 16, 4, 64
WIN = 512
SQ = 512          # queries per core
SK = 1024         # k-range per core (halo + chunk)
NCHUNK = S // SQ  # 4
N_CORES = 8
EPS = 1e-6
# heads 0..AUG_HEADS-1 get a per-query exponent correction -slope*q accumulated
# into scores (rank-1 matmul) so exp() stays bounded; for flatter slopes the
# uncorrected exponent (<= slope*511 + ~8) fits comfortably in f32.
AUG_HEADS = 6
# Head order permutation (host-side, applied to wq/q_norm_w/wo): Q-head slot
# 2j+sub sits at partition offset 64*sub, which must equal its KV head's
# partition offset 64*((h//4)%2) (matmul requires equal base partitions).
PERM = [0, 4, 1, 5, 2, 6, 3, 7, 8, 12, 9, 13, 10, 14, 11, 15]

# banded spans: for k-tile kt (128 rows of kl), valid q columns [QL[kt], QR[kt])
QL = [0, 0, 0, 0, 0, 128, 256, 384]
QR = [128, 256, 384, 512, 512, 512, 512, 512]


def _alibi_slopes(n):
    ratio = 2.0 ** (-8.0 / n)
    return np.array([ratio**i for i in range(n)], dtype=np.float64)


def build_nc(for_sim=False):
    nc = bacc.Bacc(None, target_bir_lowering=False, debug=for_sim)

    feat_d = nc.dram_tensor("feat", [128, 8, SK], F32R, kind="ExternalInput")
    wq_d = nc.dram_tensor("wq", [8, 128, 8, 128], F32R, kind="ExternalInput")
    wk_d = nc.dram_tensor("wk", [128, 8, 256], F32R, kind="ExternalInput")
    wv_d = nc.dram_tensor("wv", [128, 8, 256], F32R, kind="ExternalInput")
    wo_d = nc.dram_tensor("wo", [128, 8, M], F32R, kind="ExternalInput")
    qw_d = nc.dram_tensor("qw", [128, 8], F32, kind="ExternalInput")
    kw_d = nc.dram_tensor("kw", [128, 2], F32, kind="ExternalInput")
    btab_d = nc.dram_tensor("btab", [128, H * 8], F32, kind="ExternalInput")
    qaug_d = nc.dram_tensor("qaug", [1, 16, SQ], F32R, kind="ExternalInput")
    ones_d = nc.dram_tensor("onesin", [128, 128], F32R, kind="ExternalInput")
    vone_d = nc.dram_tensor("vone", [128, 8, KV, 2], F32R, kind="ExternalInput")
    out_d = nc.dram_tensor("out", [SQ, M], F32, kind="ExternalOutput")

    with tile.TileContext(nc) as tc:
        with (
            nc.allow_low_precision("f32r matmul operands are intentional"),
            tc.tile_pool(name="const", bufs=1) as constp,
            tc.tile_pool(name="feat", bufs=1) as featp,
            tc.tile_pool(name="wbig", bufs=1) as wbigp,
            tc.tile_pool(name="wstream", bufs=2) as wsp,
            tc.tile_pool(name="acts", bufs=1) as actsp,
            tc.tile_pool(name="sexp", bufs=3) as sexpp,
            tc.tile_pool(name="sqpool", bufs=2) as sqp,
            tc.tile_pool(name="small", bufs=1) as smallp,
            tc.tile_pool(name="outsb", bufs=2) as outp,
        ):
            # ---- constants / inputs ----
            feat_sb = featp.tile([128, 8, SK], F32R)
            for c in range(4):
                nc.sync.dma_start(feat_sb[:, 2 * c:2 * c + 2, :],
                                  feat_d[:, 2 * c:2 * c + 2, :])
            wk_sb = constp.tile([128, 8, 256], F32R)
            nc.sync.dma_start(wk_sb[:], wk_d[:])
            wv_sb = constp.tile([128, 8, 256], F32R)
            nc.sync.dma_start(wv_sb[:], wv_d[:])
            wo_sb = wbigp.tile([128, 8, M], F32R)
            for c in range(4):
                nc.sync.dma_start(wo_sb[:, 2 * c:2 * c + 2, :],
                                  wo_d[:, 2 * c:2 * c + 2, :])
            qw_sb = constp.tile([128, 8], F32)
            nc.sync.dma_start(qw_sb[:], qw_d[:])
            kw_sb = constp.tile([128, 2], F32)
            nc.sync.dma_start(kw_sb[:], kw_d[:])
            btab_sb = constp.tile([128, H * 8], F32)
            nc.sync.dma_start(btab_sb[:], btab_d[:])
            qaug_sb = constp.tile([1, 16, SQ], F32R)
            nc.sync.dma_start(qaug_sb[:], qaug_d[:])

            ones_sb = constp.tile([128, 128], F32R)
            nc.sync.dma_start(ones_sb[:], ones_d[:])
            eps_sb = constp.tile([128, 1], F32)
            nc.vector.memset(eps_sb[:], EPS)

            # ---- K projection + rmsnorm (KT [kv_hd, SK] as [128, 2, SK]) ----
            kt_sb = actsp.tile([128, 2, SK], F32R)
            with (
                tc.tile_pool(name="psK", bufs=2, space=bass.MemorySpace.PSUM) as psK,
                tc.tile_pool(name="psSK", bufs=1, space=bass.MemorySpace.PSUM) as psSK,
                tc.tile_pool(name="psBK", bufs=1, space=bass.MemorySpace.PSUM) as psBK,
            ):
                ssqk = psSK.tile([1, SK], F32)
                for t in range(2):
                    kp = psK.tile([128, SK], F32)
                    for half in range(2):
                        cs = slice(512 * half, 512 * half + 512)
                        for mt in range(8):
                            nc.tensor.matmul(
                                kp[:, cs],
                                wk_sb[:, mt, 128 * t:128 * t + 128],
                                feat_sb[:, mt, cs],
                                start=(mt == 0), stop=(mt == 7))
                    for half in range(2):
                        cs = slice(512 * half, 512 * half + 512)
                        sqk = sqp.tile([128, 512], F32R, tag="sqk")
                        nc.scalar.activation(sqk[:], kp[:, cs], AF.Square)
                        nc.tensor.matmul(ssqk[0:1, cs], ones_sb[:, 0:1],
                                         sqk[:],
                                         start=(t == 0), stop=(t == 1))
                    # kraw = k * k_norm_w  (per-partition scale)
                    nc.scalar.activation(kt_sb[:, t, :], kp[:], AF.Copy,
                                         scale=kw_sb[:, t:t + 1])
                # sqrt(mean + eps) per column; invert AFTER the broadcast
                srtk = smallp.tile([1, SK], F32R, tag="srtk")
                nc.scalar.activation(srtk[:], ssqk[:], AF.Sqrt,
                                     scale=1.0 / 256.0, bias=eps_sb[0:1, :])
                bck = psBK.tile([128, SK], F32)
                for half in range(2):
                    cs = slice(512 * half, 512 * half + 512)
                    nc.tensor.matmul(bck[:, cs], ones_sb[0:1, :],
                                     srtk[0:1, cs], start=True, stop=True)
                bck_sb = smallp.tile([128, SK], F32, tag="bck")
                nc.vector.reciprocal_approx_fast(bck_sb[:], bck[:])
                for t in range(2):
                    nc.vector.tensor_mul(kt_sb[:, t, :], kt_sb[:, t, :], bck_sb[:])

            # ---- V projection (natural [s, kv_hd] layout, with ones cols) ----
            # v_sb[p, st, g, 0] = 1, [.., 1:65] = V, [.., 65] = 1
            v_sb = actsp.tile([128, 8, KV, 66], F32R)
            nc.sync.dma_start(v_sb[:, :, :, 0:1], vone_d[:, :, :, 0:1])
            nc.sync.dma_start(v_sb[:, :, :, 65:66], vone_d[:, :, :, 1:2])
            with tc.tile_pool(name="psV", bufs=3, space=bass.MemorySpace.PSUM) as psV:
                for st in range(8):
                    vp = psV.tile([128, 256], F32)
                    for mt in range(8):
                        nc.tensor.matmul(
                            vp[:],
                            feat_sb[:, mt, 128 * st:128 * st + 128],
                            wv_sb[:, mt, :],
                            start=(mt == 0), stop=(mt == 7))
                    nc.scalar.activation(
                        v_sb[:, st, :, 1:65],
                        vp[:].rearrange("p (g d) -> p g d", g=KV),
                        AF.Copy)

            # ---- Q projection + rmsnorm (QT [hd, SQ] as [128, 8, SQ]) ----
            qt_sb = actsp.tile([128, 8, SQ], F32R)
            with (
                tc.tile_pool(name="psQ", bufs=3, space=bass.MemorySpace.PSUM) as psQ,
                tc.tile_pool(name="psSQ", bufs=1, space=bass.MemorySpace.PSUM) as psSQ,
                tc.tile_pool(name="psBQ", bufs=1, space=bass.MemorySpace.PSUM) as psBQ,
            ):
                ssqq = psSQ.tile([1, SQ], F32)
                for t in range(8):
                    wqt = wsp.tile([128, 8, 128], F32R, tag="wqt")
                    nc.sync.dma_start(wqt[:], wq_d[t])
                    qp = psQ.tile([128, SQ], F32)
                    for mt in range(8):
                        nc.tensor.matmul(qp[:], wqt[:, mt, :],
                                         feat_sb[:, mt, 512:1024],
                                         start=(mt == 0), stop=(mt == 7))
                    sqq = sqp.tile([128, SQ], F32R, tag="sqk")
                    nc.scalar.activation(sqq[:], qp[:], AF.Square)
                    nc.tensor.matmul(ssqq[:], ones_sb[:, 0:1], sqq[:],
                                     start=(t == 0), stop=(t == 7))
                    nc.scalar.activation(qt_sb[:, t, :], qp[:], AF.Copy,
                                         scale=qw_sb[:, t:t + 1])
                srtq = smallp.tile([1, SQ], F32R, tag="srtk")
                nc.scalar.activation(srtq[:], ssqq[:], AF.Sqrt,
                                     scale=1.0 / 1024.0, bias=eps_sb[0:1, :])
                bcq = psBQ.tile([128, SQ], F32)
                nc.tensor.matmul(bcq[:], ones_sb[0:1, :], srtq[0:1, :],
                                 start=True, stop=True)
                bcq_sb = smallp.tile([128, SQ], F32, tag="bck")
                nc.vector.reciprocal_approx_fast(bcq_sb[:], bcq[:])
                for t in range(8):
                    nc.vector.tensor_mul(qt_sb[:, t, :], qt_sb[:, t, :], bcq_sb[:])

            # ---- attention (per head pair), banded sliding window ----
            attn_sb = actsp.tile([128, 8, SQ], F32R)
            KT_ORDER = [3, 0, 1, 2, 4, 5, 6, 7]
            with (
                tc.tile_pool(name="psS", bufs=2, space=bass.MemorySpace.PSUM) as psS,
                tc.tile_pool(name="psPV", bufs=2, space=bass.MemorySpace.PSUM) as psPV,
                tc.tile_pool(name="psBC", bufs=1, space=bass.MemorySpace.PSUM) as psBC,
            ):
                for pair in range(8):
                    pv_e = psPV.tile([128, SQ], F32, tag="pve")
                    pv_o = psPV.tile([128, SQ], F32, tag="pvo")
                    for sub in range(2):
                        h = PERM[2 * pair + sub]
                        g = h // 4
                        po = 64 * (g % 2)
                        qo = 64 * sub
                        assert po == qo
                        for i, kt in enumerate(KT_ORDER):
                            ql, qr = QL[kt], QR[kt]
                            span = qr - ql
                            sp = psS.tile([128, SQ], F32, tag="scores")
                            aug = h < AUG_HEADS
                            nc.tensor.matmul(
                                sp[:, 0:span],
                                kt_sb[po:po + 64, g // 2,
                                         128 * kt:128 * kt + 128],
                                qt_sb[qo:qo + 64, pair, ql:qr],
                                start=True, stop=not aug)
                            if aug:
                                nc.tensor.matmul(
                                    sp[:, 0:span],
                                    ones_sb[0:1, 0:128],
                                    qaug_sb[0:1, h, ql:qr],
                                    start=False, stop=True)
                            se = sexpp.tile([128, SQ], F32R, tag="sexp")
                            c = h * 8 + kt
                            nc.scalar.activation(se[:, 0:span], sp[:, 0:span],
                                                 AF.Exp,
                                                 bias=btab_sb[:, c:c + 1],
                                                 scale=0.125)
                            # triangle masks AFTER exp: replace invalid
                            # elements (possibly inf) with 0 on GpSimd
                            if kt <= 3:  # right tri at cols [128kt,+128):
                                # keep col' <= p  (p - col' >= 0)
                                nc.gpsimd.affine_select(
                                    se[:, 128 * kt:128 * kt + 128],
                                    se[:, 128 * kt:128 * kt + 128],
                                    pattern=[[-1, 128]],
                                    compare_op=mybir.AluOpType.is_ge,
                                    fill=0.0, base=0, channel_multiplier=1)
                            else:  # left tri at cols [0:128): keep col' >= p
                                nc.gpsimd.affine_select(
                                    se[:, 0:128], se[:, 0:128],
                                    pattern=[[1, 128]],
                                    compare_op=mybir.AluOpType.is_ge,
                                    fill=0.0, base=0, channel_multiplier=-1)
                            # [V | ones]: rows 0:64 = PV, row 64 = denom
                            # (matmul out must start at partition 0)
                            pv = pv_e if sub == 0 else pv_o
                            nc.tensor.matmul(
                                pv[0:65, ql:qr],
                                v_sb[:, kt, g, 1:66],
                                se[:, 0:span],
                                start=(i == 0), stop=(i == 7))
                    # normalize: attn = pv / denom. Broadcast the raw denoms
                    # via PE, invert once (wide) with the fast approx.
                    den = smallp.tile([128, 2, SQ], F32R, tag="den")
                    nc.vector.tensor_copy(den[64:65, 0, :], pv_e[64:65, :])
                    nc.vector.tensor_copy(den[64:65, 1, :], pv_o[64:65, :])
                    bc = psBC.tile([64, 2, SQ], F32)
                    nc.tensor.matmul(bc[0:64, 0, :], ones_sb[64:65, 0:64],
                                     den[64:65, 0, :], start=True, stop=True)
                    nc.tensor.matmul(bc[0:64, 1, :], ones_sb[64:65, 0:64],
                                     den[64:65, 1, :], start=True, stop=True)
                    bc_inv = smallp.tile([64, 2, SQ], F32, tag="bcsb")
                    nc.vector.reciprocal_approx_fast(bc_inv[:], bc[:])
                    nc.vector.tensor_mul(attn_sb[0:64, pair, :],
                                         pv_e[0:64, :], bc_inv[:, 0, :])
                    attn_tmp = smallp.tile([64, SQ], F32R, tag="attn_tmp")
                    nc.vector.tensor_mul(attn_tmp[:],
                                         pv_o[0:64, :], bc_inv[:, 1, :])
                    # partition shift 0:64 -> 64:128 via SBUF-to-SBUF DMA
                    nc.sync.dma_start(attn_sb[64:128, pair, :], attn_tmp[:])

            # ---- output projection: out[s, m] ----
            with tc.tile_pool(name="psO", bufs=3, space=bass.MemorySpace.PSUM) as psO:
                for st in range(4):
                    osb = outp.tile([128, M], F32)
                    for mh in range(2):
                        op = psO.tile([128, 512], F32)
                        for ht in range(8):
                            nc.tensor.matmul(
                                op[:],
                                attn_sb[:, ht, 128 * st:128 * st + 128],
                                wo_sb[:, ht, 512 * mh:512 * mh + 512],
                                start=(ht == 0), stop=(ht == 7))
                        nc.vector.tensor_copy(osb[:, 512 * mh:512 * mh + 512],
                                              op[:])
                    nc.sync.dma_start(
                        out_d.rearrange("(st p) m -> st p m", p=128)[st],
                        osb[:])

    if for_sim:
        nc.compile()
    else:
        nc.finalize()
    return nc


def make_in_maps(features, wq, wk, wv, wo, q_norm_w, k_norm_w):
    features = np.asarray(features, np.float32)
    wq = np.asarray(wq, np.float32)
    wk = np.asarray(wk, np.float32)
    wv = np.asarray(wv, np.float32)
    wo = np.asarray(wo, np.float32)
    q_norm_w = np.asarray(q_norm_w, np.float32)
    k_norm_w = np.asarray(k_norm_w, np.float32)

    # permute Q-head order (see PERM) in wq rows, q_norm_w, wo columns
    wq_p = wq.reshape(H, D, M)[PERM].reshape(H * D, M)
    qnw_p = q_norm_w.reshape(H, D)[PERM].reshape(H * D)
    wo_tp = wo.T.reshape(H, D, M)[PERM].reshape(H * D, M)  # wo.T rows = hd

    wq_pre = np.ascontiguousarray(
        wq_p.T.reshape(8, 128, 8, 128).transpose(2, 1, 0, 3))
    # wq_p.T is [m, hd]; [mt,p,ht,c] -> want [ht, p(m%128), mt, c]
    wk_pre = np.ascontiguousarray(wk.T.reshape(8, 128, 256).transpose(1, 0, 2))
    wv_pre = np.ascontiguousarray(wv.T.reshape(8, 128, 256).transpose(1, 0, 2))
    wo_pre = np.ascontiguousarray(wo_tp.reshape(8, 128, M).transpose(1, 0, 2))
    qw_pre = np.ascontiguousarray(qnw_p.reshape(8, 128).T)
    kw_pre = np.ascontiguousarray(k_norm_w.reshape(2, 128).T)

    slopes = _alibi_slopes(H)
    p = np.arange(128)
    qaug = np.zeros((1, 16, SQ), np.float32)
    qi = np.arange(SQ, dtype=np.float64)
    for h in range(AUG_HEADS):
        qaug[0, h, :] = -8.0 * slopes[h] * qi


    in_maps = []
    for b in range(B):
        for c in range(NCHUNK):
            q0 = c * SQ
            lo, hi = q0 - WIN, q0 + SQ
            fs = np.zeros((SK, M), np.float32)
            src_lo = max(lo, 0)
            fs[src_lo - lo:, :] = features[b, src_lo:hi, :]
            feat_pre = np.ascontiguousarray(
                fs.T.reshape(8, 128, SK).transpose(1, 0, 2))
            btab = np.zeros((128, H * 8), np.float32)
            for h in range(H):
                for kt in range(8):
                    if c == 0 and kt < 4:
                        btab[:, h * 8 + kt] = -1e30
                    else:
                        btab[:, h * 8 + kt] = slopes[h] * (128 * kt + p - 512)
            in_maps.append({
                "feat": feat_pre, "wq": wq_pre, "wk": wk_pre, "wv": wv_pre,
                "wo": wo_pre, "qw": qw_pre, "kw": kw_pre,
                "btab": btab, "qaug": qaug,
                "onesin": np.ones((128, 128), np.float32),
                "vone": np.ones((128, 8, KV, 2), np.float32),
            })
    return in_maps


_NC_CACHE = {}


def kernel(features, wq, wk, wv, wo, q_norm_w, k_norm_w,
           num_heads=16, num_kv_heads=4, head_dim=64, sliding_window=512,
           **_unused):
    assert int(num_heads) == H and int(num_kv_heads) == KV
    assert int(head_dim) == D and int(sliding_window) == WIN
    from concourse.bass_utils import run_bass_kernel_spmd

    if "nc" not in _NC_CACHE:
        _NC_CACHE["nc"] = build_nc(for_sim=False)
    nc = _NC_CACHE["nc"]
    in_maps = make_in_maps(features, wq, wk, wv, wo, q_norm_w, k_norm_w)
    res = run_bass_kernel_spmd(nc, in_maps, core_ids=list(range(N_CORES)))
    outs = [r["out"] for r in res.results]
    full = np.stack(outs, axis=0).reshape(B, NCHUNK * SQ, M)
    return full.astype(np.float32)
